# revision 2
# baseline (speedup 1.0000x reference)
"""FDGNN (gnn_message_passing) Trainium2 kernel, 8-core SPMD. v2: piece-pipelined.

Strategy (v2):
- Only 3 of the reference's 6 convs feed the output:
    s1 = conv_i2s(xi0); i2 = conv_s2i(s1); s3 = conv_i2s(i2); out = tanh(s3@wo+bo)
- mlp_m commutes with the per-edge gather, so the message MLP runs per *node*.
- Destination nodes are sharded across the 8 cores.
- The AllGather of message-table rows is split into 4 PIECES along the source
  rows (pieces = window blocks [24,24,24,26]). Each piece's AG fires as soon
  as that piece's mlp_u -> mlp_m tail is done, overlapping the rest of the
  segment-sum. Gather chunks are re-keyed piece-major so a chunk's gathers
  only depend on one AG piece (8*piece_rows <= 32768 keeps int16 indices).
- All state (xT, aggT, weights, drel, iota) in bf16; psum stays fp32.
- y and out writes staged in SBUF, one DMA per piece.
- psum->aggT copies run on the Scalar engine (DVE is S-build-bound).
"""

import numpy as np
import os as _os

NCORES = 8
NNODE = 100000  # both NS and NI
PERCORE = NNODE // NCORES  # 12500
NW = 98  # windows per core (98*128 = 12544)
PADPER = NW * 128  # 12544 padded rows per core
D = 64
HM = 32
HU = 16

# pieces along the row/window axis; the last piece is small so the final
# AG piece (the only serialized one at conv boundaries) is cheap
PWIN = [28, 28, 28, 14]  # windows per piece
PW0 = [0, 28, 56, 84]  # first window of each piece
PROWS = [w * 128 for w in PWIN]  # rows per piece
PR0 = [w * 128 for w in PW0]  # first row of each piece
NCHUNK = 4  # == number of pieces; chunk cc gathers from table piece cc
CHUNK_ROWS = [NCORES * r for r in PROWS]  # [28672, 28672, 28672, 14336]
Q32 = 32  # cell padding quantum (sub-tile); tiles stay 128 slots

GT = int(_os.environ.get("KGT", "12"))  # tiles per dma_gather call
NQUEUES = int(_os.environ.get("KNQ", "4"))  # SWDGE queues (1-4)
SCRATCH = int(_os.environ.get("KSCRATCH", "65536"))
SB = int(_os.environ.get("KSB", "8"))  # tiles per S-build batch
PREP = int(_os.environ.get("KPREP", "0"))  # prepare_only + trigger_dma gathers
GB = int(_os.environ.get("KGB", "3"))  # gather pool bufs

TRACE = False  # set by test harness to capture an NTFF profile
LAST_RESULT = None  # BassKernelResults of the most recent run


# ---------------------------------------------------------------- host prep

def _prep_relation(src, dst):
    """Route edges (dst-sharded) into per-core, per-chunk gather streams.

    Chunk cc of core p's stream gathers from table piece cc, whose rows are
    [8 cores x PROWS[cc]]: row = src_core * PROWS[cc] + (src_pos - PR0[cc]).
    """
    E = src.shape[0]
    src = src.astype(np.int64)
    dst = dst.astype(np.int64)

    p = dst // PERCORE
    dl = dst - p * PERCORE
    sp = src // PERCORE  # source core
    pos = src - sp * PERCORE  # row within source core (< 12500)
    c = np.minimum(pos // PROWS[0], 3)  # piece/chunk of the source row
    lidx = sp * np.array(PROWS)[c] + (pos - np.array(PR0)[c])
    w = dl >> 7
    drel = dl - (w << 7)

    key = (p * NCHUNK + c) * NW + w
    counts = np.bincount(key, minlength=NCORES * NCHUNK * NW).reshape(
        NCORES, NCHUNK, NW
    )
    # cells padded to Q32-slot quanta (core-independent: max over cores).
    # Matmuls always use all 128 partitions; foreign slots in a shared tile
    # are masked by a per-(tile, window) incidence S column (drel = -1).
    cell_n = -(-counts.max(axis=0) // Q32) * Q32  # [NCHUNK, NW]
    cell_n[0] = np.maximum(cell_n[0], Q32)  # every window has >=1 range
    base_w = np.zeros((NCHUNK, NW + 1), np.int64)
    base_w[:, 1:] = np.cumsum(cell_n, axis=1)
    stream_n = base_w[:, -1]  # slots per chunk stream
    T_c = (-(-stream_n // 128)).astype(np.int64)  # tiles per chunk stream

    # per-window matmul incidences: (chunk, tile) pairs covering cell (c, w);
    # n_inc[cc] counts incidences per chunk (defines the drel column stream)
    win_tiles = []  # [w] -> list of (cc, t)
    n_inc = np.zeros(NCHUNK, np.int64)
    for ww in range(NW):
        rr = []
        for cc in range(NCHUNK):
            s0 = int(base_w[cc, ww])
            s1 = s0 + int(cell_n[cc, ww])
            for t in range(s0 // 128, (s1 + 127) // 128):
                rr.append((cc, t))
                n_inc[cc] += 1
        win_tiles.append(rr)

    # rank of each edge within its (p, c, w) cell
    order = np.argsort(key, kind="stable")
    kk = key[order]
    grp_first = np.r_[True, kk[1:] != kk[:-1]]
    first_pos = np.flatnonzero(grp_first)
    starts = np.repeat(first_pos, np.diff(np.r_[first_pos, E]))
    rank = np.arange(E) - starts
    inv = np.empty(E, np.int64)
    inv[order] = rank
    slot = base_w[c, w] + inv  # slot within (core, chunk) stream

    import ml_dtypes

    # incidence enumeration per chunk, in the order the conv loop consumes it
    inc_list = [[] for _ in range(NCHUNK)]  # [cc] -> (t, lo, hi) stream slots
    for ww in range(NW):
        for cc, t in win_tiles[ww]:
            s0 = int(base_w[cc, ww])
            s1 = s0 + int(cell_n[cc, ww])
            inc_list[cc].append((t, max(s0, t * 128), min(s1, (t + 1) * 128)))

    idx_streams = []  # [core][chunk] -> int16 [128, T_c*8] packed
    drel_streams = []  # [core][chunk] -> bf16 [128, n_inc] incidence columns
    for pp in range(NCORES):
        rows_i = []
        rows_d = []
        pm = p == pp
        for cc in range(NCHUNK):
            n = int(T_c[cc]) * 128
            ar = np.arange(n)
            # pad slots gather spread-out (finite) rows; S row is 0 for them
            idx_flat = (ar * 131) % CHUNK_ROWS[cc]
            drel_flat = np.full(n, -1.0, np.float32)
            m = pm & (c == cc)
            idx_flat[slot[m]] = lidx[m]
            drel_flat[slot[m]] = drel[m]
            assert idx_flat.max() < CHUNK_ROWS[cc] and idx_flat.min() >= 0
            assert CHUNK_ROWS[cc] <= 32768
            idx16 = idx_flat.astype(np.int16)
            packed = np.tile(idx16.reshape(n // 16, 16).T, (8, 1))  # [128, n/16]
            rows_i.append(np.ascontiguousarray(packed))
            cols = np.full((int(n_inc[cc]), 128), -1.0, np.float32)
            for j, (t, lo, hi) in enumerate(inc_list[cc]):
                cols[j, lo - t * 128 : hi - t * 128] = drel_flat[lo:hi]
            rows_d.append(
                np.ascontiguousarray(cols.T.astype(ml_dtypes.bfloat16))
            )
        idx_streams.append(rows_i)
        drel_streams.append(rows_d)

    return {
        "win_tiles": win_tiles,  # [NW] -> [(chunk, tile)]
        "n_inc": n_inc,  # [NCHUNK] incidence counts
        "T_c": T_c,  # [NCHUNK]
        "idx": idx_streams,
        "drel": drel_streams,
    }


# ---------------------------------------------------------------- program

def _build_program(meta_a, meta_b):
    """meta_a: i2s relation (convs 1 and 3), meta_b: s2i relation (conv 2)."""
    import concourse.mybir as mybir
    import concourse.tile as tile
    from concourse import bacc
    import ml_dtypes

    FP32 = mybir.dt.float32
    BF16 = mybir.dt.bfloat16
    I16 = mybir.dt.int16
    AF = mybir.ActivationFunctionType

    nc = bacc.Bacc(
        "TRN2",
        target_bir_lowering=False,
        debug=False,
        enable_asserts=False,
        num_devices=NCORES,
        num_swdge_queues=NQUEUES,
        dynamic_dma_scratch_size=SCRATCH,
    )

    # ---- I/O
    xi0T = nc.dram_tensor("xi0T", [D, PADPER], BF16, kind="ExternalInput")
    wm1 = nc.dram_tensor("wm1", [D, HM], BF16, kind="ExternalInput")
    bm1 = nc.dram_tensor("bm1", [HM, 1], FP32, kind="ExternalInput")
    wm2b = nc.dram_tensor("wm2b", [HM + 1, D], BF16, kind="ExternalInput")
    wu1 = nc.dram_tensor("wu1", [D, HU], BF16, kind="ExternalInput")
    bu1 = nc.dram_tensor("bu1", [HU, 1], FP32, kind="ExternalInput")
    wu2 = nc.dram_tensor("wu2", [HU, D], BF16, kind="ExternalInput")
    bu2 = nc.dram_tensor("bu2", [D, 1], FP32, kind="ExternalInput")
    wob = nc.dram_tensor("wob", [D + 1, D], BF16, kind="ExternalInput")

    idx_in = {}
    drel_in = {}
    for rel, meta in (("a", meta_a), ("b", meta_b)):
        for cc in range(NCHUNK):
            tcn = int(meta["T_c"][cc])
            idx_in[rel, cc] = nc.dram_tensor(
                f"idx_{rel}{cc}", [128, tcn * 8], I16, kind="ExternalInput"
            )
            drel_in[rel, cc] = nc.dram_tensor(
                f"drel_{rel}{cc}",
                [128, int(meta["n_inc"][cc])],
                BF16,
                kind="ExternalInput",
            )

    out = nc.dram_tensor("out", [PADPER, D], FP32, kind="ExternalOutput")

    # collective buffers per (set, piece); rows hold the 64 bf16 features
    # twice (256B gather granule)
    y_piece = [
        [nc.dram_tensor(f"y{s}_{p}", [PROWS[p], 2 * D], BF16) for p in range(4)]
        for s in range(2)
    ]
    tables = [
        [
            nc.dram_tensor(
                f"table{s}_{p}", [CHUNK_ROWS[p], 2 * D], BF16, addr_space="Shared"
            )
            for p in range(4)
        ]
        for s in range(2)
    ]

    iota_np = np.tile(
        np.arange(128, dtype=np.float32).astype(ml_dtypes.bfloat16), (128, SB, 1)
    )
    iota_dram = nc.inline_tensor(iota_np.reshape(128, SB * 128), name="iota")
    ones_dram = nc.inline_tensor(
        np.ones((1, max(PROWS)), ml_dtypes.bfloat16), name="onesrow"
    )

    def col_tiles(p):
        """512-col tiles covering piece p."""
        ts_ = [(i * 512, 512) for i in range(PROWS[p] // 512)]
        if PROWS[p] % 512:
            ts_.append((PROWS[p] - PROWS[p] % 512, PROWS[p] % 512))
        return ts_

    with tile.TileContext(nc) as tc:
        with (
            tc.tile_pool(name="consts", bufs=1) as cs,
            tc.tile_pool(name="state", bufs=1) as st,
            tc.tile_pool(name="stage", bufs=3) as sg,
            tc.tile_pool(name="ysb", bufs=2) as yp,
            tc.tile_pool(name="osb", bufs=1) as op_,
            tc.tile_pool(name="meta", bufs=2) as mp,
            tc.tile_pool(name="g0", bufs=GB) as gp0,
            tc.tile_pool(name="g1", bufs=GB) as gp1,
            tc.tile_pool(name="g2", bufs=GB) as gp2,
            tc.tile_pool(name="g3", bufs=GB) as gp3,
            tc.tile_pool(name="spool", bufs=2) as sp,
            tc.tile_pool(name="pw", bufs=2, space="PSUM") as pw,
            tc.tile_pool(name="pa", bufs=2, space="PSUM") as pa,
            tc.tile_pool(name="pb", bufs=2, space="PSUM") as pb,
            tc.tile_pool(name="pu", bufs=2, space="PSUM") as pu,
        ):
            gpools = [gp0, gp1, gp2, gp3]

            # ---- constants
            iota_s = cs.tile([128, SB, 128], BF16)
            nc.sync.dma_start(
                out=iota_s[:], in_=iota_dram[:, :].rearrange("p (b w) -> p b w", b=SB)
            )
            wm1_s = cs.tile([D, HM], BF16)
            nc.sync.dma_start(out=wm1_s[:], in_=wm1[:, :])
            bm1_s = cs.tile([HM, 1], FP32)
            nc.sync.dma_start(out=bm1_s[:], in_=bm1[:, :])
            wm2b_s = cs.tile([HM + 1, D], BF16)
            nc.sync.dma_start(out=wm2b_s[:], in_=wm2b[:, :])
            wu1_s = cs.tile([D, HU], BF16)
            nc.sync.dma_start(out=wu1_s[:], in_=wu1[:, :])
            bu1_s = cs.tile([HU, 1], FP32)
            nc.sync.dma_start(out=bu1_s[:], in_=bu1[:, :])
            wu2_s = cs.tile([HU, D], BF16)
            nc.sync.dma_start(out=wu2_s[:], in_=wu2[:, :])
            bu2_s = cs.tile([D, 1], FP32)
            nc.sync.dma_start(out=bu2_s[:], in_=bu2[:, :])
            wob_s = cs.tile([D + 1, D], BF16)
            nc.sync.dma_start(out=wob_s[:], in_=wob[:, :])

            # ---- persistent state, split per piece
            xT = []
            aggT = []
            for p in range(4):
                xt = st.tile([D + 1, PROWS[p]], BF16, tag=f"xT{p}")
                nc.sync.dma_start(
                    out=xt[0:D, :], in_=xi0T[:, PR0[p] : PR0[p] + PROWS[p]]
                )
                nc.sync.dma_start(
                    out=xt[D : D + 1, :], in_=ones_dram[:, 0 : PROWS[p]]
                )
                xT.append(xt)
                agg_t = st.tile([D, PROWS[p]], BF16, tag=f"agg{p}")
                aggT.append(agg_t)

            # h1 tiles with a persistent ones row (bias via lhsT trick)
            h1s = []
            for i in range(3):
                h1 = st.tile([HM + 1, 512], BF16, tag=f"h1_{i}")
                nc.sync.dma_start(
                    out=h1[HM : HM + 1, :], in_=ones_dram[:, 0:512]
                )
                h1s.append(h1)
            h1_rr = [0]  # round-robin cursor

            def mlp_u_piece(p):
                """xT[p] = relu(wu2.T @ relu(wu1.T @ aggT[p] + bu1) + bu2)."""
                for c0, cn in col_tiles(p):
                    ps1 = pu.tile([D, 512], FP32, tag="pu")
                    nc.tensor.matmul(
                        ps1[0:HU, :cn],
                        wu1_s[:],
                        aggT[p][:, c0 : c0 + cn],
                        start=True,
                        stop=True,
                    )
                    hu = sg.tile([HU, 512], BF16, tag="hu")
                    nc.scalar.activation(
                        hu[:, :cn], ps1[0:HU, :cn], AF.Relu, bias=bu1_s[:]
                    )
                    ps2 = pu.tile([D, 512], FP32, tag="pu")
                    nc.tensor.matmul(
                        ps2[:, :cn], wu2_s[:], hu[:, :cn], start=True, stop=True
                    )
                    nc.scalar.activation(
                        xT[p][0:D, c0 : c0 + cn], ps2[:, :cn], AF.Relu, bias=bu2_s[:]
                    )

            def mlp_m_piece(p, yset):
                """y_piece[yset][p] = mlp_m(xT[p]) rows, then AG into table."""
                ysb = yp.tile([128, PWIN[p], 2 * D], BF16, tag="ysb")
                for c0, cn in col_tiles(p):
                    psa = pa.tile([HM, 512], FP32, tag="pa")
                    nc.tensor.matmul(
                        psa[:, :cn],
                        wm1_s[:],
                        xT[p][0:D, c0 : c0 + cn],
                        start=True,
                        stop=True,
                    )
                    h1 = h1s[h1_rr[0] % 3]
                    h1_rr[0] += 1
                    nc.scalar.activation(
                        h1[0:HM, :cn], psa[:, :cn], AF.Relu, bias=bm1_s[:]
                    )
                    for j0 in range(0, cn, 128):
                        wrel = (c0 + j0) // 128
                        psb = pb.tile([128, D], FP32, tag="pb")
                        nc.tensor.matmul(
                            psb[:],
                            h1[:, j0 : j0 + 128],
                            wm2b_s[:],
                            start=True,
                            stop=True,
                        )
                        nc.scalar.activation(
                            ysb[:, wrel, 0:D], psb[:], AF.Relu
                        )
                        nc.scalar.activation(
                            ysb[:, wrel, D : 2 * D], psb[:], AF.Relu
                        )
                nc.sync.dma_start(
                    out=y_piece[yset][p][:, :].rearrange("(w i) f -> i w f", i=128),
                    in_=ysb[:, 0 : PWIN[p], :],
                )
                nc.gpsimd.collective_compute(
                    "AllGather",
                    mybir.AluOpType.bypass,
                    replica_groups=[list(range(NCORES))],
                    ins=[y_piece[yset][p].ap().opt()],
                    outs=[tables[yset][p].ap().opt()],
                )

            def h2o_piece(p):
                """out piece = tanh(x @ wo + bo) via the lhsT flip trick."""
                osb = op_.tile([128, PWIN[p], D], FP32, tag="osb")
                for wrel in range(PWIN[p]):
                    psb = pb.tile([128, D], FP32, tag="pb")
                    nc.tensor.matmul(
                        psb[:],
                        xT[p][:, wrel * 128 : (wrel + 1) * 128],
                        wob_s[:],
                        start=True,
                        stop=True,
                    )
                    nc.scalar.activation(osb[:, wrel, :], psb[:], AF.Tanh)
                nc.sync.dma_start(
                    out=out[PR0[p] : PR0[p] + PROWS[p], :].rearrange(
                        "(w i) f -> i w f", i=128
                    ),
                    in_=osb[:, 0 : PWIN[p], :],
                )

            dma_sems = [
                nc.alloc_semaphore(f"swdge_dma{q}") for q in range(NQUEUES)
            ]

            def conv(meta, rel, tset, tails, chunk_major=False):
                """Gather + segment-sum into aggT; run tails[p] at piece ends.

                chunk_major: sweep windows once per chunk, accumulating into
                aggT in SBUF (ACT copy for chunk 0, DVE add after). Lets the
                first sweep start as soon as AG piece 0 lands (used for the
                first conv, whose AGs have nothing to hide behind).
                """
                win_tiles = meta["win_tiles"]
                n_inc = meta["n_inc"]
                T_c = meta["T_c"]

                drel_s = []
                for cc in range(NCHUNK):
                    nin = int(n_inc[cc])
                    dt_ = mp.tile([128, nin], BF16, tag=f"drel{cc}")
                    nc.sync.dma_start(out=dt_[:], in_=drel_in[rel, cc][:, :])
                    drel_s.append(dt_)

                calls = []
                for cc in range(NCHUNK):
                    tcn = int(T_c[cc])
                    calls.append(
                        [(t0, min(GT, tcn - t0)) for t0 in range(0, tcn, GT)]
                    )

                gbufs = [None] * NCHUNK
                gcall = [-1] * NCHUNK
                sbufs = [None] * NCHUNK
                sbatch = [-1] * NCHUNK
                call_rr = [0]  # rotate queues when one chunk runs alone

                def ensure_gather(cc, t):
                    k = t // GT
                    if gcall[cc] != k:
                        t0, nt = calls[cc][k]
                        ix = mp.tile([128, nt * 8], I16, tag=f"idx{cc}")
                        nc.sync.dma_start(
                            out=ix[:],
                            in_=idx_in[rel, cc][:, t0 * 8 : (t0 + nt) * 8],
                        )
                        gb = gpools[cc].tile([128, nt, 2 * D], BF16, tag=f"gb{cc}")
                        q = call_rr[0] % NQUEUES if chunk_major else cc % NQUEUES
                        call_rr[0] += 1
                        if PREP:
                            nc.gpsimd.dma_gather(
                                gb[:],
                                tables[tset][cc][:, :],
                                ix[:],
                                nt * 128,
                                nt * 128,
                                2 * D,
                                elem_step=2 * D,
                                queue_num=q,
                                single_packet=GT <= 8,
                                prepare_only=True,
                                sem=dma_sems[q],
                            )
                            nc.gpsimd.trigger_dma(count=None, queue_num=q)
                        else:
                            nc.gpsimd.dma_gather(
                                gb[:],
                                tables[tset][cc][:, :],
                                ix[:],
                                nt * 128,
                                nt * 128,
                                2 * D,
                                elem_step=2 * D,
                                queue_num=q,
                                single_packet=GT <= 8,
                            )
                        gbufs[cc] = gb
                        gcall[cc] = k
                    return gbufs[cc], t - calls[cc][k][0]

                def ensure_s(cc, i):
                    """S column for incidence i of chunk cc (batched by SB)."""
                    k = i // SB
                    if sbatch[cc] != k:
                        i0 = k * SB
                        nb = min(SB, int(n_inc[cc]) - i0)
                        stile = sp.tile([128, SB, 128], BF16, tag=f"sb{cc}")
                        nc.vector.tensor_tensor(
                            out=stile[:, 0:nb, :],
                            in0=drel_s[cc][:, i0 : i0 + nb].to_broadcast(
                                [128, nb, 128]
                            ),
                            in1=iota_s[:, 0:nb, :],
                            op=mybir.AluOpType.is_equal,
                        )
                        sbufs[cc] = stile
                        sbatch[cc] = k
                    return sbufs[cc], i - k * SB

                inc_cnt = [0] * NCHUNK  # per-chunk incidence cursor

                def do_mms(ps, mms):
                    n = len(mms)
                    for j, (cc, t) in enumerate(mms):
                        gb, gslot = ensure_gather(cc, t)
                        stile, sslot = ensure_s(cc, inc_cnt[cc])
                        inc_cnt[cc] += 1
                        nc.tensor.matmul(
                            ps[:],
                            gb[:, gslot, 0:D],
                            stile[:, sslot, :],
                            start=(j == 0),
                            stop=(j == n - 1),
                        )

                if chunk_major:
                    for cc in range(NCHUNK):
                        pcur = 0
                        for w in range(NW):
                            mms = [r for r in win_tiles[w] if r[0] == cc]
                            p = pcur
                            if w == PW0[pcur] + PWIN[pcur] - 1:
                                pcur += 1
                            wrel = w - PW0[p]
                            dst = aggT[p][:, wrel * 128 : (wrel + 1) * 128]
                            if not mms:
                                if cc == 0:
                                    nc.vector.memset(dst, 0.0)
                            else:
                                ps = pw.tile([D, 128], FP32, tag="pw")
                                do_mms(ps, mms)
                                if cc == 0:
                                    nc.scalar.activation(dst, ps[:], AF.Copy)
                                else:
                                    nc.vector.tensor_tensor(
                                        out=dst,
                                        in0=dst,
                                        in1=ps[:],
                                        op=mybir.AluOpType.add,
                                    )
                            if cc == NCHUNK - 1 and w == PW0[p] + PWIN[p] - 1:
                                tails(p)
                    return

                pcur = 0
                for w in range(NW):
                    ps = pw.tile([D, 128], FP32, tag="pw")
                    do_mms(ps, win_tiles[w])
                    wrel = w - PW0[pcur]
                    nc.scalar.activation(
                        aggT[pcur][:, wrel * 128 : (wrel + 1) * 128],
                        ps[:],
                        AF.Copy,
                    )
                    if w == PW0[pcur] + PWIN[pcur] - 1:
                        tails(pcur)
                        pcur += 1

            # ---------------- the 3 convs, piece-pipelined
            for p in range(4):
                mlp_m_piece(p, 0)  # initial tables from xi0

            def tail1(p):
                mlp_u_piece(p)
                mlp_m_piece(p, 1)

            def tail2(p):
                mlp_u_piece(p)
                mlp_m_piece(p, 0)

            def tail3(p):
                mlp_u_piece(p)
                h2o_piece(p)

            conv(meta_a, "a", 0, tail1)
            conv(meta_b, "b", 1, tail2)
            conv(meta_a, "a", 0, tail3)

    nc.compile()
    return nc


# ---------------------------------------------------------------- entry

def _prepare(
    x_served,
    x_interfered,
    edge_s2i,
    edge_i2s,
    wm1,
    bm1,
    wm2,
    bm2,
    wu1,
    bu1,
    wu2,
    bu2,
    wo,
    bo,
):
    """Host prep + program build. Returns (nc, in_maps)."""
    import ml_dtypes

    BF = ml_dtypes.bfloat16
    x_interfered = np.asarray(x_interfered, np.float32)
    e_s2i = np.asarray(edge_s2i)
    e_i2s = np.asarray(edge_i2s)

    # relation a: i2s (src interfered, dst served) -- convs 1 and 3
    meta_a = _prep_relation(e_i2s[0], e_i2s[1])
    # relation b: s2i (src served, dst interfered) -- conv 2
    meta_b = _prep_relation(e_s2i[0], e_s2i[1])

    nc = _build_program(meta_a, meta_b)

    wm2b = np.concatenate(
        [np.asarray(wm2, np.float32), np.asarray(bm2, np.float32)[None, :]], axis=0
    )
    wob = np.concatenate(
        [np.asarray(wo, np.float32), np.asarray(bo, np.float32)[None, :]], axis=0
    )

    in_maps = []
    for p in range(NCORES):
        xi_loc = np.zeros((D, PADPER), np.float32)
        xi_loc[:, :PERCORE] = x_interfered[p * PERCORE : (p + 1) * PERCORE].T
        m = {
            "xi0T": xi_loc.astype(BF),
            "wm1": np.ascontiguousarray(np.asarray(wm1, np.float32).astype(BF)),
            "bm1": np.ascontiguousarray(np.asarray(bm1, np.float32).reshape(HM, 1)),
            "wm2b": wm2b.astype(BF),
            "wu1": np.ascontiguousarray(np.asarray(wu1, np.float32).astype(BF)),
            "bu1": np.ascontiguousarray(np.asarray(bu1, np.float32).reshape(HU, 1)),
            "wu2": np.ascontiguousarray(np.asarray(wu2, np.float32).astype(BF)),
            "bu2": np.ascontiguousarray(np.asarray(bu2, np.float32).reshape(D, 1)),
            "wob": wob.astype(BF),
        }
        for rel, meta in (("a", meta_a), ("b", meta_b)):
            for cc in range(NCHUNK):
                m[f"idx_{rel}{cc}"] = meta["idx"][p][cc]
                m[f"drel_{rel}{cc}"] = meta["drel"][p][cc]
        in_maps.append(m)

    return nc, in_maps


def kernel(**inputs):
    from concourse.bass_utils import run_bass_kernel_spmd

    nc, in_maps = _prepare(**inputs)
    res = run_bass_kernel_spmd(
        nc, in_maps, core_ids=list(range(NCORES)), trace=TRACE
    )
    global LAST_RESULT
    LAST_RESULT = res
    outs = [res.results[p]["out"][:PERCORE] for p in range(NCORES)]
    return np.concatenate(outs, axis=0)
